# revision 6
# baseline (speedup 1.0000x reference)
"""Coordinate multi-strip attention (pooling) kernel for 8 TRN2 NeuronCores.

Full inputs in, full outputs out. Data-parallel over batch B=32 -> 4
samples per core; all parameters replicated.

Algebraic folding done on host (all linear, exact up to fp reassociation):
  strip = mean_w(x)                      (raw sum; /64 folded into K)
  u     = (strip + dw3(strip) + dw7(strip)) / 3   (7-tap per-channel conv)
  u_bn  = (u - mean)*gamma/sqrt(var+eps) + beta   (affine per channel)
  y     = conv1_w @ concat(u_bn_h, u_bn_w)        (1x1 conv, contraction over C)
=>  y[m,l] = sum_{c,d} K[m,c,d] * strip_raw[c,l+d] + yb[m]
with K[m,c,d] = conv1_w[m,c] * wcomb[c,d] * bn_scale[c] / 64 and the bias
terms folded into the BN1 affine. The TensorEngine computes this as 7
shifted matmuls per channel-half (batched over sample pairs, N=128),
accumulating in PSUM.

Engine assignment for the big streaming passes (GPSIMD contends with
2-port DVE ops for the shared SBUF port, so they are interleaved with
care):
  ScalarE: transposed copy of each x tile so both strip reductions read
           dense (strided DVE reduce costs 7.0us vs 4.4us dense)
  DVE    : dense strip reductions + late g-builds (+ last tile's multiply)
  GPSIMD : final x*g multiplies + early g-builds
X tiles are loaded in h-halves so the first reduction starts ~8us earlier.
"""

import numpy as np

import concourse.bass as bass
import concourse.mybir as mybir
import concourse.tile as tile
from concourse import bacc
from concourse.bass_utils import run_bass_kernel_spmd

EPS = 1e-5
F32 = mybir.dt.float32
N_CORES = 8
B_LOCAL = 4  # 32 / 8
C = 256
H = 64
W = 64

# Per (b, cb): (g_build_engine, multiply_engine); 'v' = DVE, 'g' = gpsimd.
# Early g-builds go to GPSIMD (idle early); late ones to DVE (free late).
_GATE_PLAN = {
    (0, 0): ('g', 'g'), (0, 1): ('g', 'g'),
    (1, 0): ('g', 'g'), (1, 1): ('v', 'g'),
    (2, 0): ('v', 'g'), (2, 1): ('v', 'g'),
    (3, 0): ('v', 'g'), (3, 1): ('v', 'v'),
}

_CACHE = {}


def _build_program():
    from contextlib import ExitStack

    nc = bacc.Bacc(
        "TRN2",
        target_bir_lowering=False,
        debug=False,
        enable_asserts=True,
        num_devices=N_CORES,
    )

    x_d = nc.dram_tensor("x", [B_LOCAL, C, H, W], F32, kind="ExternalInput")
    kt_d = nc.dram_tensor("kt", [2, 2, 128, 56], F32, kind="ExternalInput")
    wgt_d = nc.dram_tensor("wgt", [2, 8, 256], F32, kind="ExternalInput")
    sb_d = nc.dram_tensor("sb", [8, 8], F32, kind="ExternalInput")
    out_d = nc.dram_tensor("out", [B_LOCAL, C, H, W], F32, kind="ExternalOutput")

    mult = mybir.AluOpType.mult
    Relu = mybir.ActivationFunctionType.Relu
    Identity = mybir.ActivationFunctionType.Identity
    Sigmoid = mybir.ActivationFunctionType.Sigmoid
    Copy = mybir.ActivationFunctionType.Copy

    with tile.TileContext(nc) as tc, ExitStack() as ctx:
        const = ctx.enter_context(tc.tile_pool(name="const", bufs=1))
        xpool = ctx.enter_context(tc.tile_pool(name="xp", bufs=8))
        xtpool = ctx.enter_context(tc.tile_pool(name="xt", bufs=2))
        gpool = ctx.enter_context(tc.tile_pool(name="gp", bufs=2))
        strips = ctx.enter_context(tc.tile_pool(name="strips", bufs=1))
        vpool = ctx.enter_context(tc.tile_pool(name="vp", bufs=2))
        apool = ctx.enter_context(tc.tile_pool(name="ap", bufs=8))
        psum_y = ctx.enter_context(tc.tile_pool(name="py", bufs=2, space="PSUM"))
        psum_q = ctx.enter_context(tc.tile_pool(name="pq", bufs=2, space="PSUM"))
        psum_g = ctx.enter_context(tc.tile_pool(name="pg", bufs=4, space="PSUM"))

        # Constants
        kt_t = {}
        for dd in range(2):
            for cb in range(2):
                t = const.tile([128, 56], F32, tag=f"kt{dd}{cb}")
                nc.sync.dma_start(out=t[:], in_=kt_d[dd, cb])
                kt_t[dd, cb] = t
        wgt_t = {}
        for dd in range(2):
            t = const.tile([8, 256], F32, tag=f"wgt{dd}")
            nc.sync.dma_start(out=t[:], in_=wgt_d[dd])
            wgt_t[dd] = t
        sb_t = const.tile([8, 8], F32, tag="sb")
        nc.sync.dma_start(out=sb_t[:], in_=sb_d[:])

        # Strip tensors: [128c, 4b, 70] with 3-wide zero pads on both ends
        strip_t = {}
        for dd in range(2):
            for cb in range(2):
                t = strips.tile([128, B_LOCAL, 70], F32, tag=f"st{dd}{cb}")
                nc.gpsimd.memset(t[:, :, 0:3], 0.0)
                nc.gpsimd.memset(t[:, :, 67:70], 0.0)
                strip_t[dd, cb] = t

        X = {}
        A = {}
        for pair in range(2):
            bs = (2 * pair, 2 * pair + 1)
            for b in bs:
                for cb in range(2):
                    t = xpool.tile([128, H, W], F32, tag="X")
                    # load in h-halves: first reduction can start earlier
                    for hh in range(2):
                        nc.sync.dma_start(
                            out=t[:, hh * 32:(hh + 1) * 32],
                            in_=x_d[b, cb * 128:(cb + 1) * 128,
                                    hh * 32:(hh + 1) * 32],
                        )
                    X[b, cb] = t
                for cb in range(2):
                    xt = xtpool.tile([128, W, H], F32, tag="XT")
                    for hh in range(2):
                        hs = slice(hh * 32, (hh + 1) * 32)
                        # dense w-reduce per h-half straight off x
                        nc.vector.reduce_sum(
                            out=strip_t[0, cb][:, b, 3 + hh * 32:3 + (hh + 1) * 32],
                            in_=X[b, cb][:, hs],
                            axis=mybir.AxisListType.X,
                        )
                        # ScalarE transposes the half; h-reduce reads dense
                        nc.scalar.activation(
                            out=xt[:, :, hs].rearrange("p w h -> p h w"),
                            in_=X[b, cb][:, hs],
                            func=Copy,
                        )
                    nc.vector.reduce_sum(
                        out=strip_t[1, cb][:, b, 3:67],
                        in_=xt[:],
                        axis=mybir.AxisListType.X,
                    )

            # y_pre for the pair: 7 shifted matmuls x 2 channel halves,
            # batched over the 2 samples (N=128), PSUM-accumulated.
            yp = {}
            for dd in range(2):
                p = psum_y.tile([8, 2, 64], F32, tag="yp")
                n_mm = 0
                for cb in range(2):
                    for di in range(7):
                        nc.tensor.matmul(
                            p[:],
                            lhsT=kt_t[dd, cb][:, di * 8:(di + 1) * 8],
                            rhs=strip_t[dd, cb][:, 2 * pair:2 * pair + 2,
                                                di:di + 64],
                            start=(n_mm == 0),
                            stop=(n_mm == 13),
                        )
                        n_mm += 1
                yp[dd] = p

            # BN1 + hswish:  z = s1*yp + b1_dir;  v = z * min(relu(z+3), 6)
            # q lives in PSUM so the min/mul avoid the SBUF port entirely.
            q = psum_q.tile([8, 2, 2, 64], F32, tag="q")  # [m, b2, dir, 64]
            v = vpool.tile([8, 2, 2, 64], F32, tag="v")
            for dd in range(2):
                nc.scalar.activation(
                    out=q[:, :, dd], in_=yp[dd][:], func=Relu,
                    scale=sb_t[:, 0:1], bias=sb_t[:, 3 + dd:4 + dd],
                )
                nc.scalar.activation(
                    out=v[:, :, dd], in_=yp[dd][:], func=Identity,
                    scale=sb_t[:, 0:1], bias=sb_t[:, 1 + dd:2 + dd],
                )
            nc.vector.tensor_scalar_min(q[:], q[:], 6.0)
            nc.vector.tensor_mul(v[:], v[:], q[:])

            # Gates: a = sigmoid(Wg/6 @ v), batched over the pair (N=128)
            for dd in range(2):
                for cb in range(2):
                    ga = psum_g.tile([128, 2, 64], F32, tag="ga")
                    nc.tensor.matmul(
                        ga[:],
                        lhsT=wgt_t[dd][:, cb * 128:(cb + 1) * 128],
                        rhs=v[:, :, dd],
                        start=True,
                        stop=True,
                    )
                    at = apool.tile([128, 2, 64], F32, tag="a")
                    nc.scalar.activation(out=at[:], in_=ga[:], func=Sigmoid)
                    A[pair, dd, cb] = at

            # g = a_h (x) a_w ; X *= g ; store
            for b in bs:
                ip = b - 2 * pair
                for cb in range(2):
                    g_eng, m_eng = _GATE_PLAN[b, cb]
                    g = gpool.tile([128, H, W], F32, tag="g")
                    ah_ap = A[pair, 0, cb][:, ip]  # [128, 64]
                    aw_ap = A[pair, 1, cb][:, ip]
                    ah = ah_ap.broadcast_to([128, H, W])  # [c, h, w*]
                    aw = bass.AP(
                        aw_ap.tensor, aw_ap.offset,
                        [list(aw_ap.ap[0]), [0, H], list(aw_ap.ap[1])],
                    )  # [c, h*, w]
                    eng1 = nc.vector if g_eng == 'v' else nc.gpsimd
                    eng2 = nc.vector if m_eng == 'v' else nc.gpsimd
                    eng1.tensor_tensor(g[:], ah, aw, mult)
                    eng2.tensor_tensor(X[b, cb][:], X[b, cb][:], g[:], mult)
                    nc.sync.dma_start(
                        out=out_d[b, cb * 128:(cb + 1) * 128], in_=X[b, cb][:]
                    )

    nc.compile()
    return nc


def _fold_strip_params(w3, w7, gamma, beta, mean, var):
    scale = gamma / np.sqrt(var + EPS)  # [C]
    wc = np.zeros((C, 7), np.float64)
    wc[:, 3] += 1.0
    wc[:, 2:5] += w3.astype(np.float64)
    wc[:, :] += w7.astype(np.float64)
    wc /= 3.0
    Wt = wc * scale[:, None].astype(np.float64) / 64.0  # [C, 7]
    bias_c = beta - mean * scale  # [C]
    return Wt, bias_c


def _pack_params(inp):
    conv1 = inp["conv1_w"].astype(np.float64)  # [8, 256]
    kt = np.zeros((2, 2, 128, 56), np.float32)
    sb = np.zeros((8, 8), np.float32)
    s1 = inp["bn1_gamma"] / np.sqrt(inp["bn1_var"] + EPS)  # [8]

    for dd, pre in enumerate(("sph", "spw")):
        Wt, bias_c = _fold_strip_params(
            inp[f"{pre}_w3"], inp[f"{pre}_w7"], inp[f"{pre}_gamma"],
            inp[f"{pre}_beta"], inp[f"{pre}_mean"], inp[f"{pre}_var"],
        )
        K = conv1[:, :, None] * Wt[None, :, :]  # [8, 256, 7]
        for cb in range(2):
            blk = K[:, cb * 128:(cb + 1) * 128, :]  # [8, 128, 7]
            kt[dd, cb] = blk.transpose(1, 2, 0).reshape(128, 56).astype(np.float32)
        yb = conv1 @ bias_c  # [8]
        b1 = (yb - inp["bn1_mean"]) * s1 + inp["bn1_beta"]  # [8]
        sb[:, 1 + dd] = b1.astype(np.float32)
        sb[:, 3 + dd] = (b1 + 3.0).astype(np.float32)

    sb[:, 0] = s1.astype(np.float32)

    wgt = np.zeros((2, 8, 256), np.float32)
    wgt[0] = (inp["convh_w"].T / 6.0).astype(np.float32)  # [m, o]
    wgt[1] = (inp["convw_w"].T / 6.0).astype(np.float32)
    return kt, wgt, sb


def kernel(**inputs):
    if "nc" not in _CACHE:
        _CACHE["nc"] = _build_program()
    nc = _CACHE["nc"]

    x = np.ascontiguousarray(inputs["x"], dtype=np.float32)
    kt, wgt, sb = _pack_params(inputs)

    in_maps = []
    for i in range(N_CORES):
        in_maps.append({
            "x": x[i * B_LOCAL:(i + 1) * B_LOCAL],
            "kt": kt,
            "wgt": wgt,
            "sb": sb,
        })
    res = run_bass_kernel_spmd(nc, in_maps, list(range(N_CORES)))
    out = np.concatenate([res.results[i]["out"] for i in range(N_CORES)], axis=0)
    return out
